# revision 24
# baseline (speedup 1.0000x reference)
"""Trainium2 Bass kernel for nn_CrossAttentionExpert.

Problem (hardcoded shapes): B=4, C=256, H=W=64 (N=4096), C8=32.
  cross_p2v = attn(q=wq_p@f_p, k=wk_v@f_v, v=wv_v@f_v)
  cross_v2p = attn(q=wq_v@f_v, k=wk_p@f_p, v=wv_p@f_p)
  out = BN(w_out @ concat([f_p, f_v, cross_p2v, cross_v2p]))  (training BN)

Sharding: 8 cores = (batch b, spatial half h).  Each core computes both
attention directions for its 2048 query positions (keys span all 4096
positions of its batch), plus BN with a [128,4] AllReduce of per-channel
sum/sumsq.

Measured: ~300-312us HW exec (vs 695us for the all-f32r predecessor),
rel err 8.6e-3 (gate 2e-2).  Run-to-run spread is ~±6% from the PE power
throttle (K=13/16 / K=4/8 clock states).  Engine budget at steady state:
PE ~200us (pacer; AV matmuls are 134us of it at the throttled ~1.95GHz),
ACT ~150us (64 exps of [128,2048] at the 1.2GHz 1-elem/lane rate),
DVE ~200us (softmax denominators + PSUM->SBUF casts; capped by the TRN2
SBUF-src errata at ~1.15 elem/cycle), tail ~70us (drain + ~36us NRT
collective floor + BN + writeback).

Design notes:
- All matmuls bf16 (1 col/cycle, FWL weight loads, less power throttle);
  inputs cast to bf16 host-side (halves DMA).
- Cross-term output conv folded into V host-side (wv' = w_out[:,cross]@wv)
  so AV directly produces y contributions; V-bias dropped entirely (it
  shifts y by a per-channel constant which training-mode BN cancels
  exactly); 1/rowsum applied to the folded 256-ch AV output.
- Scores S^T (keys on partitions feed AV with no transposes); the K=32
  contraction packed 4x via tile_position row tiling, with kt/qr
  replicated across partition groups for free by replicating the tiny
  conv weights 4x along stationary columns (the 4 concurrent matmuls
  share one moving-operand stream).
- Two-deep software pipeline over (dir, mtile): ACT exps tile t while the
  PE runs tile t-1's AV matmuls + spread-out conv "filler" work; PSUM =
  4-bank score group + 2-bank AV accumulator + 2 misc banks.
- Softmax denominator: contiguous bf16 pairwise adds (DVE) +
  gpsimd.partition_all_reduce (sum over key-partitions + broadcast) +
  reciprocal_approx_fast.  The av-scale of tile t-1 is emitted *before*
  tile t's denominator chain so the AV PSUM banks free without stalling
  the PE at mtile boundaries.
"""

import numpy as np
import ml_dtypes

import concourse.bass as bass
import concourse.bass_isa as bass_isa
import concourse.mybir as mybir
import concourse.tile as tile
from concourse import bacc, bass_utils

FP = mybir.dt.float32
BF = mybir.dt.bfloat16
P = 128
C = 256
C8 = 32
N = 4096          # keys per batch
M = 2048          # local query positions per core
NMT = 4           # m-tiles of 512
MT = 512
NCH = 32          # key chunks of 128 per m-tile
NGR = 8           # groups of 4 key chunks
NCORES = 8
BN_EPS = 1e-5
BN_COUNT = 4 * 4096

_ALU = mybir.AluOpType
_ACT = mybir.ActivationFunctionType

_PROGRAM = None

# BN-stats allreduce implementation: remote-DMA XOR tree (3 hops on the
# intra-chip links) vs the NRT collective stack (~36us incl. barrier).
# The rdma path compiles (raw post-tile block; Tile's scheduler cannot
# model peer-satisfied semaphores) but hits NRT_EXEC_UNIT_UNRECOVERABLE
# on this stack — left disabled as documentation of the attempt.
USE_RDMA_STATS = False


def _build_program():
    nc = bacc.Bacc("TRN2", target_bir_lowering=False, debug=False,
                   num_devices=NCORES)

    # ---- DRAM I/O ----
    # kv0 = f_v (rolled), kv1 = f_p (rolled), bf16
    kv = [nc.dram_tensor(f"kv{d}", [C, N], BF, kind="ExternalInput").ap()
          for d in range(2)]
    # all bf16 weights packed host-side into one [128, 3072] tensor
    # (each dma_start costs ~640ns of Sync-engine dispatch; one big load
    # beats nine small ones).  Layout along free axis:
    #   wq0 | wk0 | wq1 | wk1        (4 x [p, 2, 128])
    #   wv0 | wv1                    (2 x [p, 2, 256])
    #   wdir                         (1 x [p, 4, 256])
    wpack = nc.dram_tensor("wpack", [P, 3072], BF, kind="ExternalInput").ap()
    # fp32 consts: qkbias (4 cols) | gamma/beta (4 cols)
    fpc = nc.dram_tensor("fpc", [P, 8], FP, kind="ExternalInput").ap()
    yout = nc.dram_tensor("y", [C, M], FP, kind="ExternalOutput").ap()

    if USE_RDMA_STATS:
        ar_rsem = [nc.alloc_semaphore(f"ar_rsem{s}") for s in range(3)]
        ar_lsem = nc.alloc_semaphore("ar_lsem")
        s_prep = nc.alloc_semaphore("s_prep")
        s_astep = nc.alloc_semaphore("s_astep")
        s_sync0 = nc.alloc_semaphore("s_sync0")
        s_vsem = nc.alloc_semaphore("s_vsem")
        s_ssem = nc.alloc_semaphore("s_ssem")
        s_wsem = nc.alloc_semaphore("s_wsem")
        s_odsem = nc.alloc_semaphore("s_odsem")
        _all_sems = ar_rsem + [ar_lsem, s_prep, s_astep, s_sync0, s_vsem,
                               s_ssem, s_wsem, s_odsem]

    with tile.TileContext(nc) as tc:
        with (
            tc.tile_pool(name="consts", bufs=1) as consts,
            tc.tile_pool(name="big", bufs=1) as big,
            tc.tile_pool(name="kt", bufs=2) as p_kt,
            tc.tile_pool(name="qr", bufs=2) as p_qr,
            tc.tile_pool(name="vt", bufs=2) as p_vt,
            tc.tile_pool(name="stg", bufs=2) as p_stg,
            tc.tile_pool(name="row", bufs=2) as p_row,
            tc.tile_pool(name="small", bufs=4) as p_small,
            tc.tile_pool(name="ps4", bufs=1, space="PSUM") as ps4p,
            tc.tile_pool(name="psav", bufs=1, space="PSUM") as psavp,
            tc.tile_pool(name="psm", bufs=2, space="PSUM") as psm,
            tc.tile_pool(name="dram", bufs=1, space="DRAM") as dram,
        ):
            if USE_RDMA_STATS:
                # sems are not cleared by allocation and persist across NEFF
                # executions; reset before any peer's sends can land (peers
                # send ~280us into their own run, we clear at ~0)
                for s in _all_sems:
                    nc.gpsimd.sem_clear(s)

            # ---- load constants first (small; conv matmuls need them
            # ---- before the big kv tensors finish) ----
            wpack_sb = consts.tile([P, 3072], BF, name="wpack_sb")
            nc.sync.dma_start(wpack_sb[:], wpack[:])
            fpc_sb = consts.tile([P, 8], FP, name="fpc_sb")
            nc.sync.dma_start(fpc_sb[:], fpc[:])

            def wview(off, o, m):
                return wpack_sb[:, off:off + o * m].rearrange(
                    "p (o m) -> p o m", o=o)

            wq_sb = [wview(512 * d, 2, P) for d in range(2)]
            wk_sb = [wview(512 * d + 256, 2, P) for d in range(2)]
            wv_sb = [wview(1024 + 512 * d, 2, C) for d in range(2)]
            wdir_sb = wview(2048, 4, C)
            qkb_sb = fpc_sb[:, 0:4]
            gb_sb = fpc_sb[:, 4:8]

            kv_sb = []
            srcs = []
            for d in range(2):
                t = big.tile([P, 2, N], BF, name=f"kvsb{d}")
                srcs.append(kv[d].rearrange("(o p) n -> p o n", p=P))
                kv_sb.append(t)
            # first-half columns of both tensors land first (the dir0 q/k
            # convs in the prologue touch both kv tensors right away)
            for h in range(2):
                hs = slice(h * 2048, (h + 1) * 2048)
                for d in range(2):
                    for o in range(2):
                        nc.sync.dma_start(kv_sb[d][:, o, hs],
                                          srcs[d][:, o, hs])

            y_acc = [big.tile([P, M], FP, name=f"yacc{cc}") for cc in range(2)]
            stats_s = big.tile([P, 8], FP, name="stats_s")
            stats_q = big.tile([P, 8], FP, name="stats_q")
            scr = big.tile([P, MT], BF, name="scr")  # discard target

            # ---- per-dir persistent tiles (allocated up front; pools give
            # ---- each dir its own buffer) ----
            qr_t = [p_qr.tile([P, M], BF, tag="qr", name=f"qr{d}")
                    for d in range(2)]
            kt_t = [p_kt.tile([P, N], BF, tag="kt", name=f"kt{d}")
                    for d in range(2)]
            vt_t = [p_vt.tile([P, NCH, C], BF, tag="vt", name=f"vt{d}")
                    for d in range(2)]

            # ---- conv work units (each: a few matmuls + one DVE op) ----
            def unit_direct(oc, t):
                def emit():
                    msl = slice(t * MT, (t + 1) * MT)
                    ocs = slice(oc * P, (oc + 1) * P)
                    ps = psm.tile([P, MT], FP, tag="misc")
                    for j, (kvi, o) in enumerate(
                            ((1, 0), (1, 1), (0, 0), (0, 1))):
                        nc.tensor.matmul(ps, wdir_sb[:, j, ocs],
                                         kv_sb[kvi][:, o, slice(t * MT,
                                                                (t + 1) * MT)],
                                         start=(j == 0), stop=(j == 3))
                    nc.vector.tensor_copy(y_acc[oc][:, msl], ps)
                return emit

            def unit_qr(d, t):
                def emit():
                    qkv = kv_sb[1 - d]
                    msl = slice(t * MT, (t + 1) * MT)
                    ps = psm.tile([P, MT], FP, tag="misc")
                    for kc in range(2):
                        nc.tensor.matmul(ps, wq_sb[d][:, kc, :],
                                         qkv[:, kc, msl],
                                         start=(kc == 0), stop=(kc == 1))
                    nc.vector.tensor_scalar_add(
                        qr_t[d][:, msl], ps, qkb_sb[:, 2 * d:2 * d + 1])
                return emit

            def unit_kt(d, sub):
                def emit():
                    kkv = kv_sb[d]
                    nsl = slice(sub * MT, (sub + 1) * MT)
                    ps = psm.tile([P, MT], FP, tag="misc")
                    for kc in range(2):
                        nc.tensor.matmul(ps, wk_sb[d][:, kc, :],
                                         kkv[:, kc, nsl],
                                         start=(kc == 0), stop=(kc == 1))
                    nc.vector.tensor_scalar_add(
                        kt_t[d][:, nsl], ps, qkb_sb[:, 2 * d + 1:2 * d + 2])
                return emit

            def unit_vt(d, j2):
                def emit():
                    kkv = kv_sb[d]
                    ps = psm.tile([P, 2, C], FP, tag="misc")
                    for jj in range(2):
                        j = 2 * j2 + jj
                        for kc in range(2):
                            nc.tensor.matmul(
                                ps[:, jj, :],
                                kkv[:, kc, j * P:(j + 1) * P],
                                wv_sb[d][:, kc, :],
                                start=(kc == 0), stop=(kc == 1))
                    # split the PSUM->SBUF casts between DVE and ACT
                    dst = vt_t[d][:, 2 * j2:2 * j2 + 2, :]
                    if j2 % 2 == 0:
                        nc.vector.tensor_copy(dst, ps)
                    else:
                        nc.scalar.copy(dst, ps)
                return emit

            # filler schedule: tile index i = 4*d + t -> slot -> units
            fillers = {i: [[] for _ in range(NGR)] for i in range(8)}

            def spread(units, i):
                nslots = NGR
                for u, fn in enumerate(units):
                    fillers[i][(u * nslots) // len(units)].append(fn)

            spread([unit_direct(oc, t) for oc in range(2)
                    for t in range(NMT)] +
                   [unit_vt(0, j2) for j2 in range(16)], 0)
            spread([unit_qr(1, t) for t in range(NMT)] +
                   [unit_kt(1, sub) for sub in range(8)], 2)
            spread([unit_vt(1, j2) for j2 in range(8)], 3)
            spread([unit_vt(1, j2) for j2 in range(8, 16)], 4)

            # prologue: dir0 q/k convs only
            for t in range(NMT):
                unit_qr(0, t)()
            for sub in range(8):
                unit_kt(0, sub)()

            # ---- software pipeline over (dir, mtile) ----
            tiles = [(d, t) for d in range(2) for t in range(NMT)]
            prev = None  # (d, t, stg, av, rinv, msl)

            def emit_av_group(pv, g):
                d_, t_, stg_, av_, _, _ = pv
                for i in range(4):
                    ch = 4 * g + i
                    for cc in range(2):
                        nc.tensor.matmul(
                            av_[:, cc, :],
                            vt_t[d_][:, ch, cc * P:(cc + 1) * P],
                            stg_[:, ch, :],
                            start=(g == 0 and i == 0),
                            stop=(g == NGR - 1 and i == 3),
                            skip_group_check=True)

            def finish_prev(pv):
                """Scale prev tile's AV output by 1/rowsum into y_acc and,
                for dir1 tiles, fold BN partial stats."""
                d_, t_, _, av_, rinv_, msl_ = pv
                for cc in range(2):
                    tmp = p_small.tile([P, MT], FP, tag="avtmp")
                    nc.vector.tensor_mul(tmp[:], av_[:, cc, :], rinv_[:])
                    nc.vector.tensor_add(y_acc[cc][:, msl_],
                                         y_acc[cc][:, msl_], tmp[:])
                if d_ == 1:
                    for cc in range(2):
                        col = slice(cc * 4 + t_, cc * 4 + t_ + 1)
                        nc.scalar.activation(
                            scr[:], y_acc[cc][:, msl_], _ACT.Square,
                            accum_out=stats_q[:, col])
                        nc.vector.reduce_sum(stats_s[:, col],
                                             y_acc[cc][:, msl_],
                                             axis=mybir.AxisListType.X)

            for (d, t) in tiles:
                qr, kt, vt = qr_t[d], kt_t[d], vt_t[d]
                i = 4 * d + t
                msl = slice(t * MT, (t + 1) * MT)
                stg = p_stg.tile([P, NCH, MT], BF, tag="stg")
                av = psavp.tile([P, 2, MT], FP, tag="av")
                racc = p_row.tile([P, MT], FP, tag="racc")
                for g in range(NGR):
                    ps = ps4p.tile([P, 4, MT], FP, tag="ps4")
                    for q in range(4):
                        ch = 4 * g + q
                        nc.tensor.matmul(
                            ps[:, q, :],
                            kt[32 * q:32 * (q + 1), ch * P:(ch + 1) * P],
                            qr[32 * q:32 * (q + 1), msl],
                            start=True, stop=True,
                            tile_position=(32 * q, 0))
                    if prev is not None:
                        emit_av_group(prev, g)
                    for fn in fillers[i][g]:
                        fn()
                    nc.scalar.activation(stg[:, 4 * g:4 * g + 4, :], ps[:],
                                         _ACT.Exp)
                    if g == NGR - 1 and prev is not None:
                        # free prev's AV banks before this tile's denominator
                        # chain so the next tile's AV matmuls aren't stalled
                        finish_prev(prev)
                    # rowsum partials: first pairwise level on DVE (bf16
                    # double-rate), second level on GpSimd, fp32 accumulate
                    # back on DVE
                    t1 = p_small.tile([P, 2, MT], BF, tag="t1")
                    nc.vector.tensor_add(t1[:], stg[:, 4 * g:4 * g + 2, :],
                                         stg[:, 4 * g + 2:4 * g + 4, :])
                    t2 = p_small.tile([P, MT], BF, tag="t2")
                    nc.vector.tensor_add(t2[:], t1[:, 0, :], t1[:, 1, :])
                    if g == 0:
                        nc.vector.tensor_copy(racc[:], t2[:])
                    else:
                        nc.vector.tensor_add(racc[:], racc[:], t2[:])
                rbc = p_row.tile([P, MT], FP, tag="rbc")
                nc.gpsimd.partition_all_reduce(rbc[:], racc[:], P,
                                               bass_isa.ReduceOp.add)
                rinv = p_row.tile([P, MT], FP, tag="rinv")
                nc.vector.reciprocal_approx_fast(out=rinv[:], in_=rbc[:])
                prev = (d, t, stg, av, rinv, msl)

            # drain: last tile's AV + scale + stats
            for g in range(NGR):
                emit_av_group(prev, g)
            finish_prev(prev)

            # ---- BN: pack stats, AllReduce, normalize ----
            stats = p_small.tile([P, 4], FP, tag="stats")
            for cc in range(2):
                nc.vector.reduce_sum(stats[:, cc:cc + 1],
                                     stats_s[:, 4 * cc:4 * cc + 4],
                                     axis=mybir.AxisListType.X)
                nc.vector.reduce_sum(stats[:, 2 + cc:3 + cc],
                                     stats_q[:, 4 * cc:4 * cc + 4],
                                     axis=mybir.AxisListType.X)
            if USE_RDMA_STATS:
                # buffers for the raw-bass allreduce tail (allocated in the
                # tile region so SBUF space is reserved; used after it)
                accs = [stats] + [big.tile([P, 4], FP, name=f"aracc{s}")
                                  for s in range(3)]
                recvs = [big.tile([P, 4], FP, name=f"arrecv{s}")
                         for s in range(3)]
                bnmath = big.tile([P, 12], FP, name="bnmath")
            else:
                cc_in = dram.tile([P, 4], FP)
                cc_out = dram.tile([P, 4], FP)
                nc.sync.dma_start(cc_in[:], stats[:])
                nc.gpsimd.collective_compute(
                    "AllReduce", _ALU.add,
                    replica_groups=[list(range(NCORES))],
                    ins=[cc_in.opt()], outs=[cc_out.opt()])
                ar = p_small.tile([P, 4], FP, tag="ar")
                nc.sync.dma_start(ar[:], cc_out[:])

            inv_n = 1.0 / BN_COUNT
            yo = yout.rearrange("(o p) m -> p o m", p=P)
            if not USE_RDMA_STATS:
                for cc in range(2):
                    mean = p_small.tile([P, 1], FP, tag="bn")
                    ex2 = p_small.tile([P, 1], FP, tag="bn")
                    var = p_small.tile([P, 1], FP, tag="bn")
                    nc.vector.tensor_scalar_mul(mean[:], ar[:, cc:cc + 1],
                                                inv_n)
                    nc.vector.tensor_scalar_mul(ex2[:], ar[:, 2 + cc:3 + cc],
                                                inv_n)
                    nc.vector.tensor_tensor(var[:], mean[:], mean[:],
                                            _ALU.mult)
                    nc.vector.tensor_sub(var[:], ex2[:], var[:])
                    sd = p_small.tile([P, 1], FP, tag="bn")
                    nc.vector.tensor_scalar_add(var[:], var[:], BN_EPS)
                    nc.scalar.activation(sd[:], var[:], _ACT.Sqrt)
                    rstd = p_small.tile([P, 1], FP, tag="bn")
                    nc.vector.reciprocal(rstd[:], sd[:])
                    scale = p_small.tile([P, 1], FP, tag="bn")
                    nc.vector.tensor_tensor(scale[:], gb_sb[:, cc:cc + 1],
                                            rstd[:], _ALU.mult)
                    shift = p_small.tile([P, 1], FP, tag="bn")
                    nc.vector.tensor_tensor(shift[:], mean[:], scale[:],
                                            _ALU.mult)
                    nc.vector.tensor_sub(shift[:], gb_sb[:, 2 + cc:3 + cc],
                                         shift[:])
                    for q in range(2):
                        qsl = slice(q * 1024, (q + 1) * 1024)
                        nc.vector.tensor_scalar(
                            out=y_acc[cc][:, qsl], in0=y_acc[cc][:, qsl],
                            scalar1=scale[:], scalar2=shift[:],
                            op0=_ALU.mult, op1=_ALU.add)
                        nc.sync.dma_start(yo[:, cc, qsl], y_acc[cc][:, qsl])

    if USE_RDMA_STATS:
        # Raw-bass tail: XOR-tree allreduce over the 8 same-chip cores via
        # direct SBUF-to-SBUF remote DMA (rdests are relative — Q7 XORs the
        # delta-tpb with its own id, so the program stays SPMD), then BN math
        # + normalize + writeback with hand-rolled cross-engine semaphores.
        # Raw because Tile's scheduling simulator cannot model semaphores
        # satisfied by a peer core and declares a deadlock.
        def cc_(ap):
            # tile-pool APs are symbolic; post-schedule the concrete tensor
            # exists and raw-block instructions need it
            if hasattr(ap.tensor, "concrete_tensor"):
                ap = ap.clone() if hasattr(ap, "clone") else ap
                ap.tensor = ap.tensor.concrete_tensor()
            return ap

        mean, ex2, var = bnmath[:, 0:2], bnmath[:, 2:4], bnmath[:, 4:6]
        sd, rstd = bnmath[:, 6:8], bnmath[:, 8:10]
        scale, shift = bnmath[:, 10:12], bnmath[:, 4:6]  # shift reuses var
        mean, ex2, var, sd, rstd, scale, shift = map(
            cc_, (mean, ex2, var, sd, rstd, scale, shift))
        with nc.Block() as blk:

            @blk.gpsimd
            def _(g):
                g.wait_ge(s_sync0, 1)
                for s, dk in enumerate((1, 2, 4)):
                    if s > 0:
                        g.wait_ge(s_astep, s)
                    rd = [None] * 8
                    rd[dk] = (0, dk)
                    g.remote_dma_broadcast(
                        out_ap=cc_(recvs[s][:]), in_ap=cc_(accs[s][:]),
                        remote_sem=ar_rsem[s], local_sem=ar_lsem,
                        rdests=rd).then_inc(s_prep, 1)
                    g.wait_ge(s_prep, s + 1)
                    g.trigger_dma(count=1)

            @blk.vector
            def _(v):
                v.sem_inc(s_sync0, 1)
                for s in range(3):
                    v.wait_ge(ar_rsem[s], 2)
                    v.tensor_add(cc_(accs[s + 1][:]), cc_(accs[s][:]),
                                 cc_(recvs[s][:]))
                    v.sem_inc(s_astep, 1)
                ar = accs[3]
                v.tensor_scalar_mul(mean, cc_(ar[:, 0:2]), inv_n)
                v.tensor_scalar_mul(ex2, cc_(ar[:, 2:4]), inv_n)
                v.tensor_tensor(var, mean, mean, _ALU.mult)
                v.tensor_sub(var, ex2, var)
                v.tensor_scalar_add(var, var, BN_EPS)
                v.sem_inc(s_vsem, 1)
                v.wait_ge(s_ssem, 1)
                v.reciprocal(rstd, sd)
                v.tensor_tensor(scale, cc_(gb_sb[:, 0:2]), rstd, _ALU.mult)
                v.tensor_tensor(shift, mean, scale, _ALU.mult)
                v.tensor_sub(shift, cc_(gb_sb[:, 2:4]), shift)
                for cc in range(2):
                    for q in range(2):
                        qsl = slice(q * 1024, (q + 1) * 1024)
                        v.tensor_scalar(
                            out=cc_(y_acc[cc][:, qsl]),
                            in0=cc_(y_acc[cc][:, qsl]),
                            scalar1=scale[:, cc:cc + 1],
                            scalar2=shift[:, cc:cc + 1],
                            op0=_ALU.mult, op1=_ALU.add)
                        v.sem_inc(s_wsem, 1)

            @blk.scalar
            def _(sc):
                sc.wait_ge(s_vsem, 1)
                sc.activation(sd, var, _ACT.Sqrt)
                sc.sem_inc(s_ssem, 1)

            @blk.sync
            def _(sy):
                for i, (cc, q) in enumerate(
                        (c, qq) for c in range(2) for qq in range(2)):
                    qsl = slice(q * 1024, (q + 1) * 1024)
                    sy.wait_ge(s_wsem, i + 1)
                    sy.dma_start(yo[:, cc, qsl],
                                 cc_(y_acc[cc][:, qsl])).then_inc(
                                     s_odsem, 16)
                sy.wait_ge(s_odsem, 64)

    nc.compile()
    return nc


def _get_program():
    global _PROGRAM
    if _PROGRAM is None:
        _PROGRAM = _build_program()
    return _PROGRAM


def _bf(x):
    return np.ascontiguousarray(np.asarray(x, np.float32)).astype(
        ml_dtypes.bfloat16)


def _make_in_maps(inputs):
    f_p = np.ascontiguousarray(
        np.asarray(inputs["f_p"], np.float32).reshape(4, C, N))
    f_v = np.ascontiguousarray(
        np.asarray(inputs["f_v"], np.float32).reshape(4, C, N))

    w_out = np.asarray(inputs["w_out"], np.float32)

    def rep4(w):  # [32, 256] -> [256, 128] (4 col-copies of w^T)
        return np.tile(np.asarray(w, np.float32).T, (1, 4))

    def fused_v(dcol, wv_):  # (w_out[:, dcol] @ wv)^T [256, 256]
        blk = w_out[:, dcol * C:(dcol + 1) * C]
        return (blk @ np.asarray(wv_, np.float32)).T

    def tile4(b):  # [32] -> [128]
        return np.tile(np.asarray(b, np.float32), 4)

    def fold(w):  # [2^k*128, F] -> [128, 2^k * F] ((o p) m -> p (o m))
        o = w.shape[0] // P
        return w.reshape(o, P, -1).transpose(1, 0, 2).reshape(P, -1)

    # dir0 (p2v): q from f_p, k/v from f_v; dir1 (v2p): reversed
    wpack = np.concatenate(
        [fold(rep4(inputs["wq_p"])), fold(rep4(inputs["wk_v"])),
         fold(rep4(inputs["wq_v"])), fold(rep4(inputs["wk_p"])),
         fold(fused_v(2, inputs["wv_v"])), fold(fused_v(3, inputs["wv_p"])),
         fold(w_out[:, :2 * C].T)], axis=1)
    fpc = np.stack(
        [tile4(inputs["bq_p"]), tile4(inputs["bk_v"]),
         tile4(inputs["bq_v"]), tile4(inputs["bk_p"]),
         np.asarray(inputs["gamma"], np.float32)[:P],
         np.asarray(inputs["gamma"], np.float32)[P:],
         np.asarray(inputs["beta"], np.float32)[:P],
         np.asarray(inputs["beta"], np.float32)[P:]], axis=1)
    shared = {
        "wpack": _bf(wpack),
        "fpc": np.ascontiguousarray(fpc, np.float32),
    }
    in_maps = []
    for core in range(NCORES):
        b, h = divmod(core, 2)
        # roll so this core's query half sits at columns [0, 2048); K/V use
        # the full (permuted) range — softmax/AV are order-invariant in keys.
        kv1 = _bf(np.roll(f_p[b], -h * M, axis=1))
        kv0 = _bf(np.roll(f_v[b], -h * M, axis=1))
        in_maps.append({"kv0": kv0, "kv1": kv1, **shared})
    return in_maps


def _assemble(results):
    out = np.empty((4, C, N), np.float32)
    for core in range(NCORES):
        b, h = divmod(core, 2)
        out[b][:, h * M:(h + 1) * M] = results[core]["y"]
    return out.reshape(4, C, 64, 64)


def _run(inputs, **kwargs):
    nc = _get_program()
    in_maps = _make_in_maps(inputs)
    res = bass_utils.run_bass_kernel_spmd(
        nc, in_maps, core_ids=list(range(NCORES)), **kwargs)
    return _assemble(res.results), res


def kernel(**inputs):
    out, _ = _run(inputs)
    return out
